# revision 1
# baseline (speedup 1.0000x reference)
"""Trainium2 Bass kernel for nn_CustomizedSelfAttention.

Reference computation (per batch sample b):
    q = x @ Wq; k = x @ Wk; v = x @ Wv
    attn = softmax(q @ k.T * C**-0.5)          # [N, N]
    y = attn @ v @ Wp + bp + x                 # [N, C]
    pooled = mean(y, axis=0)                   # [C]
    out = relu(pooled @ Wf1 + bf1) @ Wf2 + bf2 # [C]

Key algebraic collapse (exact): only the token-mean of the attention output
is needed, so with  t[m] = sum_n softmax_row_n[m]  (column sums of the
attention matrix):
    mean_n(attn @ v) = (t/N) @ v = ((t/N) @ x) @ Wv
    pooled = ((t/N) @ x) @ (Wv @ Wp) + bp + mean_n(x)
This removes the O(N^2 C) attn@v matmul and the O(N C^2) v/p projections.
Also  q @ k.T * s = x @ (Wq @ Wk.T * s) @ x.T = x @ A @ x.T  with A fused on
the host, removing one more projection.

Per-core device work:
    xT  = transpose(x) in bf16 (PE transpose of 128x128 blocks)
    GT  = A.T-side projection: GT[c,n] = sum_c' A[c',c] xT[c',n]  (bf16)
    S   = per 128-token row-block: S[n,m] = sum_c GT[c,n] xT[c,m] (PSUM f32)
    E   = exp(S) -> SBUF bf16, with per-row sums Z via ACT accum_out
    r   = 1/Z (bf16)
    tT  = PSUM accumulator [128, 32]: tT[p,j] += E[:,128j+p] . r  (PE)
    tail (fp32): Y[c,0:2] = sum_m x[m,c]*(t[m]/N, 1/N)  -> u, xbar
         pooled = u @ Wvp + bp + xbar ; h = relu(pooled @ Wf1 + bf1)
         out = h @ Wf2 + bf2   (all as 128x128-block mat-vec chains on PE)

Sharding: data-parallel over batch, 1 sample per core, weights replicated.
"""

import numpy as np
import ml_dtypes
from contextlib import ExitStack

import concourse.bass as bass
import concourse.tile as tile
from concourse import bacc, mybir
from concourse.bass_utils import run_bass_kernel_spmd

B, N, C = 8, 4096, 896
NCORES = 8
P = 128
CCH = C // P          # 7 feature chunks of 128
NT = N // P           # 32 token tiles of 128
MCH = 512             # S free-dim chunk
NMC = N // MCH        # 8
TCH = 512             # token chunk for G projection
NTC = N // TCH        # 8
BF16 = mybir.dt.bfloat16
FP8 = mybir.dt.float8e4
F32 = mybir.dt.float32

_BF = ml_dtypes.bfloat16
_F8 = ml_dtypes.float8_e4m3

MODE = "fp8"       # "bf16" or "fp8" attention matmuls
ASC = 128.0        # fp8 scale folded into A, undone in exp's affine
CG8 = 4            # fp8 c-groups of 256 (C padded 896 -> 1024)
KNOBS = {"exp_wide": True}  # experiment switches (part of build cache key)


def _build_body(ctx: ExitStack, tc: "tile.TileContext", aps: dict, mode=MODE):
    nc = tc.nc
    x_d = aps["xc"]
    a_d = aps["abf"]
    wvp_d = aps["wvp"]
    wf1_d = aps["wf1"]
    wf2_d = aps["wf2"]
    bias_d = aps["biasR"]
    ident_d = aps["ident"]
    out_d = aps["outT"]

    const_pool = ctx.enter_context(tc.tile_pool(name="const", bufs=1))
    a_pool = ctx.enter_context(tc.tile_pool(name="a", bufs=1))
    xt_pool = ctx.enter_context(tc.tile_pool(name="xt", bufs=1))
    xin_pool = ctx.enter_context(tc.tile_pool(name="xin", bufs=6))
    gt_pool = ctx.enter_context(tc.tile_pool(name="gt", bufs=3))
    e_pool = ctx.enter_context(tc.tile_pool(name="e", bufs=3))
    small_pool = ctx.enter_context(tc.tile_pool(name="small", bufs=4))
    w_pool = ctx.enter_context(tc.tile_pool(name="w", bufs=2))
    tail_pool = ctx.enter_context(tc.tile_pool(name="tail", bufs=1))
    ps_pool = ctx.enter_context(tc.tile_pool(name="ps", bufs=3, space="PSUM"))
    ps2_pool = ps_pool
    acc_pool = ctx.enter_context(tc.tile_pool(name="acc", bufs=1, space="PSUM"))

    # --- constants ---
    ident = const_pool.tile([P, P], BF16, tag="ident")
    nc.sync.dma_start(ident[:], ident_d)
    bias_sb = const_pool.tile([P, 3 * CCH], F32, tag="bias")
    nc.sync.dma_start(bias_sb[:], bias_d)
    ones1 = const_pool.tile([P, 1], BF16, tag="ones1")
    nc.vector.memset(ones1[:], 1.0)
    if mode == "bf16":
        a_sb = a_pool.tile([P, CCH, C], BF16, tag="a")
        nc.sync.dma_start(a_sb[:], a_d.rearrange("(cc p) d -> p cc d", p=P))
    else:
        # A pre-scaled by ASC, zero-padded to 1024 rows on host.
        a_sb = a_pool.tile([P, CG8, 2, C], FP8, tag="a")
        for bb in range(2 * CG8):
            nc.sync.dma_start(a_sb[:, bb // 2, bb % 2, :],
                              a_d[bb * P:(bb + 1) * P, :])

    # --- phase 0: transpose x into xT, feature-major ---
    if mode == "bf16":
        xt = xt_pool.tile([P, CCH, N], BF16, tag="xt")
        xt_view = lambda cc: xt[:, cc, :]
    else:
        xt = xt_pool.tile([P, CG8, 2, N], FP8, tag="xt")
        nc.vector.memset(xt[:, CG8 - 1, 1, :], 0.0)  # pad features 896..1023
        xt_view = lambda cc: xt[:, cc // 2, cc % 2, :]
    for nt in range(NT):
        xinb = xin_pool.tile([P, C], BF16, tag="xinb")
        nc.gpsimd.dma_start(xinb[:], x_d[nt * P:(nt + 1) * P, :])  # casts f32->bf16
        for cc in range(CCH):
            pt = ps_pool.tile([P, P], BF16, tag="ps")
            nc.tensor.transpose(pt[:], xinb[:, cc * P:(cc + 1) * P], ident[:])
            # split the psum->sbuf copies across DVE and ACT (ACT is idle
            # during phase 0, DVE is otherwise the phase-0 bottleneck)
            eng = nc.vector if cc % 2 == 0 else nc.scalar
            if eng is nc.vector:
                eng.tensor_copy(xt_view(cc)[:, nt * P:(nt + 1) * P], pt[:])
            else:
                eng.copy(xt_view(cc)[:, nt * P:(nt + 1) * P], pt[:])

    # --- main loop ---
    GRP = 8                    # n-tiles summed (r-scaled) before colsum
    NGRP = NT // GRP
    tT = acc_pool.tile([P, NT], F32, tag="acc")  # column-sum accumulator
    pend_colsum = []           # deferred (group_idx, esum) colsum emissions

    def emit_colsum(gidx, es):
        # Single accumulation group over the whole kernel: start only on the
        # very first matmul into the bank (start=True clears the whole 2KB
        # PSUM bank), stop only on the very last.
        for j in range(NT):
            nc.tensor.matmul(
                tT[:, j:j + 1], es[:, j * P:(j + 1) * P], ones1[:],
                start=(gidx == 0 and j == 0),
                stop=(gidx == NGRP - 1 and j == NT - 1),
                skip_group_check=True,
            )

    esum = None

    DR = mybir.MatmulPerfMode.DoubleRow
    TCHM = TCH if mode == "bf16" else 2 * TCH  # token chunk for G
    for ch in range(N // TCHM):
        # G projection for this token chunk: GT[c, n]
        if mode == "bf16":
            gt = gt_pool.tile([P, CCH, TCHM], BF16, tag="gt")
            for dd in range(CCH):
                gp = ps_pool.tile([P, TCHM], F32, tag="ps")
                for cc in range(CCH):
                    nc.tensor.matmul(
                        gp[:], a_sb[:, cc, dd * P:(dd + 1) * P],
                        xt[:, cc, ch * TCHM:(ch + 1) * TCHM],
                        start=(cc == 0), stop=(cc == CCH - 1),
                    )
                nc.scalar.copy(gt[:, dd, :], gp[:])
        else:
            gt = gt_pool.tile([P, CG8, 2, TCHM], FP8, tag="gt")
            nc.vector.memset(gt[:, CG8 - 1, 1, :], 0.0)
            for dd in range(CCH):
                gp = ps2_pool.tile([P, 2, TCH], F32, tag="ps")
                for g in range(CG8):
                    for h in range(2):
                        # start/stop per h: each h-half is its own PSUM bank
                        # (2KB zero region), so each needs its own clear.
                        nc.tensor.matmul(
                            gp[:, h, :], a_sb[:, g, :, dd * P:(dd + 1) * P],
                            xt[:, g, :,
                               ch * TCHM + h * TCH:ch * TCHM + (h + 1) * TCH],
                            start=(g == 0), stop=(g == CG8 - 1),
                            perf_mode=DR, skip_group_check=True,
                        )
                nc.vector.tensor_copy(gt[:, dd // 2, dd % 2, :], gp[:])
        for q in range(TCHM // P):
            nt = ch * (TCHM // P) + q
            while pend_colsum:
                emit_colsum(*pend_colsum.pop(0))
            e_t = e_pool.tile([P, N], BF16, tag="e")
            zp = small_pool.tile([P, NMC], F32, tag="zp")

            if mode == "bf16":
                for mj in range(NMC):
                    sp = ps_pool.tile([P, MCH], F32, tag="ps")
                    for cc in range(CCH):
                        nc.tensor.matmul(
                            sp[:], gt[:, cc, q * P:(q + 1) * P],
                            xt[:, cc, mj * MCH:(mj + 1) * MCH],
                            start=(cc == 0), stop=(cc == CCH - 1),
                            skip_group_check=True,
                        )
                    nc.scalar.activation(
                        e_t[:, mj * MCH:(mj + 1) * MCH], sp[:],
                        mybir.ActivationFunctionType.Exp,
                        accum_out=zp[:, mj:mj + 1],
                    )
            else:
                # pairs of m-chunks share each DoubleRow LDWEIGHTS; one exp
                # call covers the 2-bank PSUM pair (halves ACT call overhead)
                for mj2 in range(NMC // 2):
                    sps = ps2_pool.tile([P, 2, MCH], F32, tag="ps", name="sps")
                    for g in range(CG8):
                        for k in range(2):
                            mj = 2 * mj2 + k
                            nc.tensor.matmul(
                                sps[:, k, :], gt[:, g, :, q * P:(q + 1) * P],
                                xt[:, g, :, mj * MCH:(mj + 1) * MCH],
                                start=(g == 0), stop=(g == CG8 - 1),
                                perf_mode=DR, skip_group_check=True,
                            )
                    if KNOBS["exp_wide"]:
                        nc.scalar.activation(
                            e_t[:, mj2 * 2 * MCH:(mj2 + 1) * 2 * MCH], sps[:],
                            mybir.ActivationFunctionType.Exp,
                            scale=1.0 / ASC,
                            accum_out=zp[:, mj2:mj2 + 1],
                        )
                    else:
                        for k in range(2):
                            mj = 2 * mj2 + k
                            nc.scalar.activation(
                                e_t[:, mj * MCH:(mj + 1) * MCH], sps[:, k, :],
                                mybir.ActivationFunctionType.Exp,
                                scale=1.0 / ASC,
                                accum_out=zp[:, mj:mj + 1],
                            )

            z = small_pool.tile([P, 1], F32, tag="z")
            nzp = NMC if (mode == "bf16" or not KNOBS["exp_wide"]) else NMC // 2
            zp_used = zp[:, 0:nzp]
            nc.vector.reduce_sum(z[:], zp_used, axis=mybir.AxisListType.X)
            rf = small_pool.tile([P, 1], F32, tag="rf")
            nc.vector.reciprocal(rf[:], z[:])
            # fold 1/Z into E so the colsum needs no per-tile weights
            gi, gq = nt // GRP, nt % GRP
            if gq == 0:
                esum = e_pool.tile([P, N], BF16, tag="esum")
                nc.vector.tensor_scalar_mul(esum[:], e_t[:], rf[:])
            else:
                nc.vector.tensor_scalar_mul(e_t[:], e_t[:], rf[:])
                nc.vector.tensor_add(esum[:], esum[:], e_t[:])
            if gq == GRP - 1:
                pend_colsum.append((gi, esum))
    while pend_colsum:
        emit_colsum(*pend_colsum.pop(0))

    # --- tail (fp32) ---
    # TO[:, j, 0] = t[j-th chunk]/N ; TO[:, j, 1] = 1/N
    TO = tail_pool.tile([P, NT, 2], F32, tag="to")
    nc.vector.memset(TO[:, :, 1], 1.0 / N)
    nc.scalar.mul(TO[:, :, 0], tT[:], 1.0 / N)

    uxS = tail_pool.tile([P, CCH, 2], F32, tag="ux")
    if KNOBS.get("y_wide", True):
        # Y2[row, c]: row 0 = u = (t/N)@x, row 1 = xbar. A 2-column
        # stationary operand (TO pair) with x rows as the wide moving
        # operand: 64 matmuls total instead of 224 per-block weight loads.
        Y2 = ps_pool.tile([2, C], F32, tag="ps", name="y2")
        for j in range(NT):
            xin = xin_pool.tile([P, C], F32, tag="xin")
            nc.sync.dma_start(xin[:], x_d[j * P:(j + 1) * P, :])
            for (o, w) in ((0, 512), (512, 384)):
                nc.tensor.matmul(
                    Y2[:, o:o + w], TO[:, j, :], xin[:, o:o + w],
                    start=(j == 0), stop=(j == NT - 1),
                    skip_group_check=True,
                )
        y2S = tail_pool.tile([2, C], F32, tag="y2s")
        nc.scalar.copy(y2S[:], Y2[:])
        identF2 = tail_pool.tile([2, 2], F32, tag="idf2")
        nc.vector.tensor_copy(identF2[:], ident[0:2, 0:2])
        for cc in range(CCH):
            ptx = ps_pool.tile([P, 2], F32, tag="ps", name="ptx")
            nc.tensor.transpose(ptx[:], y2S[:, cc * P:(cc + 1) * P],
                                identF2[:])
            nc.vector.tensor_copy(uxS[:, cc, :], ptx[:])
    else:
        Y = ps_pool.tile([P, CCH, 2], F32, tag="ps")  # [u | xbar] chunks
        for j in range(NT):
            xin = xin_pool.tile([P, C], F32, tag="xin")
            nc.sync.dma_start(xin[:], x_d[j * P:(j + 1) * P, :])
            for cc in range(CCH):
                nc.tensor.matmul(
                    Y[:, cc, :], xin[:, cc * P:(cc + 1) * P], TO[:, j, :],
                    start=(j == 0 and cc == 0),
                    stop=(j == NT - 1 and cc == CCH - 1),
                    skip_group_check=True,
                )
        nc.scalar.copy(uxS[:], Y[:])

    def matvec(w_d, vec_cols, out_psum):
        # out_psum[:, ee] = sum_cc W[cc-block, ee-block].T @ vec[cc]
        w_sb = w_pool.tile([P, CCH, C], F32, tag="w")
        nc.sync.dma_start(w_sb[:], w_d.rearrange("(cc p) e -> p cc e", p=P))
        for ee in range(CCH):
            for cc in range(CCH):
                nc.tensor.matmul(
                    out_psum[:, ee:ee + 1],
                    w_sb[:, cc, ee * P:(ee + 1) * P],
                    vec_cols(cc),
                    start=(cc == 0), stop=(cc == CCH - 1),
                )

    P2 = ps_pool.tile([P, CCH], F32, tag="ps")
    matvec(wvp_d, lambda cc: uxS[:, cc, 0:1], P2)
    pooledS = tail_pool.tile([P, CCH], F32, tag="pooled")
    nc.vector.tensor_add(pooledS[:], P2[:], uxS[:, :, 1])
    nc.vector.tensor_add(pooledS[:], pooledS[:], bias_sb[:, 0:CCH])

    H2 = ps_pool.tile([P, CCH], F32, tag="ps")
    matvec(wf1_d, lambda cc: pooledS[:, cc:cc + 1], H2)
    hS = tail_pool.tile([P, CCH], F32, tag="h")
    nc.vector.tensor_add(hS[:], H2[:], bias_sb[:, CCH:2 * CCH])
    nc.vector.tensor_scalar_max(hS[:], hS[:], 0.0)

    O2 = ps_pool.tile([P, CCH], F32, tag="ps")
    matvec(wf2_d, lambda cc: hS[:, cc:cc + 1], O2)
    outS = tail_pool.tile([P, CCH], F32, tag="out")
    nc.vector.tensor_add(outS[:], O2[:], bias_sb[:, 2 * CCH:3 * CCH])
    nc.sync.dma_start(out_d, outS[:])

    if "dbg_ux" in aps:
        nc.sync.dma_start(aps["dbg_ux"], uxS[:])
        nc.sync.dma_start(aps["dbg_pooled"], pooledS[:])
        nc.sync.dma_start(aps["dbg_h"], hS[:])
        to_f = tail_pool.tile([P, NT], F32, tag="dbgt")
        nc.scalar.copy(to_f[:], tT[:])
        nc.sync.dma_start(aps["dbg_t"], to_f[:])
        if mode == "bf16":
            xt_f = tail_pool.tile([P, CCH, P], F32, tag="dbgxt")
            nc.vector.tensor_copy(xt_f[:], xt[:, :, 0:P])
            nc.sync.dma_start(aps["dbg_xt"], xt_f[:])
            gt_f = tail_pool.tile([P, CCH, P], F32, tag="dbggt")
            nc.vector.tensor_copy(gt_f[:], gt[:, :, 0:P])
            nc.sync.dma_start(aps["dbg_gt"], gt_f[:])
        e_f = tail_pool.tile([P, MCH], F32, tag="dbge")
        nc.vector.tensor_copy(e_f[:], e_t[:, 0:MCH])
        nc.sync.dma_start(aps["dbg_e"], e_f[:])
        nc.sync.dma_start(aps["dbg_r"], r_t[:])


_NC_CACHE = {}


def build_nc(debug=False, reps=1, mode=MODE):
    key = ("nc", debug, reps, mode, tuple(sorted(KNOBS.items())))
    if key in _NC_CACHE:
        return _NC_CACHE[key]
    nc = bacc.Bacc(
        "TRN2", target_bir_lowering=False, debug=False,
        enable_asserts=False, num_devices=NCORES,
    )
    a_shape, a_dt = ([C, C], BF16) if mode == "bf16" else ([2 * CG8 * P, C], FP8)
    aps = {
        "xc": nc.dram_tensor("xc", [N, C], F32, kind="ExternalInput").ap(),
        "abf": nc.dram_tensor("abf", a_shape, a_dt, kind="ExternalInput").ap(),
        "wvp": nc.dram_tensor("wvp", [C, C], F32, kind="ExternalInput").ap(),
        "wf1": nc.dram_tensor("wf1", [C, C], F32, kind="ExternalInput").ap(),
        "wf2": nc.dram_tensor("wf2", [C, C], F32, kind="ExternalInput").ap(),
        "biasR": nc.dram_tensor("biasR", [P, 3 * CCH], F32, kind="ExternalInput").ap(),
        "ident": nc.dram_tensor("ident", [P, P], BF16, kind="ExternalInput").ap(),
        "outT": nc.dram_tensor("outT", [P, CCH], F32, kind="ExternalOutput").ap(),
    }
    if debug:
        dbg_list = [
            ("dbg_ux", [P, CCH, 2]), ("dbg_pooled", [P, CCH]), ("dbg_h", [P, CCH]),
            ("dbg_t", [P, NT]), ("dbg_e", [P, MCH]), ("dbg_r", [P, 1]),
        ]
        if mode == "bf16":
            dbg_list += [("dbg_xt", [P, CCH, P]), ("dbg_gt", [P, CCH, P])]
        for nm, shp in dbg_list:
            dt = BF16 if nm == "dbg_r" else F32
            aps[nm] = nc.dram_tensor(nm, shp, dt, kind="ExternalOutput").ap()
    with tile.TileContext(nc) as tc:
        for _ in range(reps):
            with ExitStack() as ctx:
                _build_body(ctx, tc, aps, mode=mode)
    nc.compile()
    _NC_CACHE[key] = nc
    return nc


def prep_in_maps(x_, Wq, Wk, Wv, Wp, bp, Wf1, bf1, Wf2, bf2, mode=MODE):
    f32 = np.float32
    x_ = np.ascontiguousarray(np.asarray(x_, dtype=f32))
    A = (np.asarray(Wq, f32) @ np.asarray(Wk, f32).T) * np.float32(C ** -0.5)
    if mode == "bf16":
        abf = np.ascontiguousarray(A.astype(_BF))
    else:
        a_pad = np.zeros((2 * CG8 * P, C), f32)
        a_pad[:C] = A * np.float32(ASC)
        abf = np.ascontiguousarray(a_pad.astype(_F8))
    wvp = np.ascontiguousarray(np.asarray(Wv, f32) @ np.asarray(Wp, f32))
    wf1 = np.ascontiguousarray(np.asarray(Wf1, f32))
    wf2 = np.ascontiguousarray(np.asarray(Wf2, f32))
    biasR = np.concatenate(
        [np.asarray(b, f32).reshape(CCH, P).T for b in (bp, bf1, bf2)], axis=1
    )
    biasR = np.ascontiguousarray(biasR)
    ident = np.eye(P, dtype=_BF)
    shared = {
        "abf": abf, "wvp": wvp, "wf1": wf1, "wf2": wf2,
        "biasR": biasR, "ident": ident,
    }
    return [dict(shared, xc=np.ascontiguousarray(x_[b])) for b in range(B)]


def assemble_output(results):
    out = np.empty((B, C), dtype=np.float32)
    for b in range(B):
        out[b] = np.asarray(results[b]["outT"], np.float32).T.reshape(C)
    return out


def kernel(**inputs) -> np.ndarray:
    nc = build_nc()
    in_maps = prep_in_maps(**inputs)
    res = run_bass_kernel_spmd(nc, in_maps, list(range(NCORES)))
    return assemble_output(res.results)


if __name__ == "__main__":
    import jax
    import reference as R
    inp = {k: np.asarray(v) for k, v in R.setup_inputs().items()}
    out = kernel(**inp)
    print(out.shape, out.dtype)



# revision 2
# speedup vs baseline: 11.1384x; 11.1384x over previous
"""Trainium2 Bass kernel for nn_CustomizedSelfAttention.

Reference computation (per batch sample b):
    q = x @ Wq; k = x @ Wk; v = x @ Wv
    attn = softmax(q @ k.T * C**-0.5)          # [N, N]
    y = attn @ v @ Wp + bp + x                 # [N, C]
    pooled = mean(y, axis=0)                   # [C]
    out = relu(pooled @ Wf1 + bf1) @ Wf2 + bf2 # [C]

Key algebraic collapse (exact): only the token-mean of the attention output
is needed, so with  t[m] = sum_n softmax_row_n[m]  (column sums of the
attention matrix):
    mean_n(attn @ v) = (t/N) @ v = ((t/N) @ x) @ Wv
    pooled = ((t/N) @ x) @ (Wv @ Wp) + bp + mean_n(x)
This removes the O(N^2 C) attn@v matmul and the O(N C^2) v/p projections.
Also  q @ k.T * s = x @ (Wq @ Wk.T * s) @ x.T = x @ A @ x.T  with A fused on
the host, removing one more projection.

Per-core device pipeline (1 sample per core, weights replicated):
  phase 0 (per 1024-token chunk, overlapped with the x DMA stream):
    xbf[:, nt, :]  <- x rows, cast f32->bf16 on DMA (persistent all kernel)
    xT             <- PE 128x128 transposes, stored fp8 (feature-major)
    GT[d, n]       <- A.T-projection of the chunk, fp8 DoubleRow matmuls
  main loop (per pair of 128-row tiles):
    S   = GT.T xT  -> PSUM f32 (fp8 DoubleRow)
    E   = exp(S/ASC) -> e2[P, 2, N] fp8, row sums Z via ACT accum_out
    r   = (N/Z) in fp8 (DVE reduce + mul + reciprocal)
    tT  += per 128-col block: E_pair[:, blk].T @ r_pair  (fp8 DoubleRow,
           moving operand = r; the softmax normalization costs no DVE pass)
  tail (short, bf16 matmuls):
    TO  = [tT/N^2 | 1/N] bf16; Y2[2, C] = sum_j TO_j.T @ xbf_j  (u and xbar)
    pooled = u @ Wvp + bp + xbar; h = relu(pooled @ Wf1 + bf1)
    out = h @ Wf2 + bf2  (weights prefetched as bf16 at kernel start)
"""

import numpy as np
import ml_dtypes
from contextlib import ExitStack

import concourse.bass as bass
import concourse.tile as tile
from concourse import bacc, mybir
from concourse.bass_utils import run_bass_kernel_spmd

B, N, C = 8, 4096, 896
NCORES = 8
P = 128
CCH = C // P          # 7 feature chunks of 128
NT = N // P           # 32 token tiles of 128
MCH = 512             # S free-dim chunk (one PSUM bank)
NMC = N // MCH        # 8
TCH = 512             # token chunk for G projection
TCHM = 2 * TCH        # tokens per G chunk (fp8 DoubleRow)
NCH = N // TCHM       # 4
BF16 = mybir.dt.bfloat16
FP8 = mybir.dt.float8e4
F32 = mybir.dt.float32

_BF = ml_dtypes.bfloat16
_F8 = ml_dtypes.float8_e4m3

ASC = 128.0        # fp8 scale folded into A, undone in exp's affine
CG8 = 4            # fp8 c-groups of 256 (C padded 896 -> 1024)
DR = mybir.MatmulPerfMode.DoubleRow


def _build_body(ctx: ExitStack, tc: "tile.TileContext", aps: dict):
    nc = tc.nc
    x_d = aps["xc"]
    a_d = aps["abf"]
    bias_d = aps["biasR"]
    ident_d = aps["ident"]
    out_d = aps["outT"]

    const_pool = ctx.enter_context(tc.tile_pool(name="const", bufs=1))
    big_pool = ctx.enter_context(tc.tile_pool(name="big", bufs=1))
    e_pool = ctx.enter_context(tc.tile_pool(name="e", bufs=2))
    small_pool = ctx.enter_context(tc.tile_pool(name="small", bufs=4))
    tail_pool = ctx.enter_context(tc.tile_pool(name="tail", bufs=1))
    ps_pool = ctx.enter_context(tc.tile_pool(name="ps", bufs=3, space="PSUM"))
    acc_pool = ctx.enter_context(tc.tile_pool(name="acc", bufs=1, space="PSUM"))

    # --- constants / prefetches ---
    ident = const_pool.tile([P, P], BF16, tag="ident")
    nc.sync.dma_start(ident[:], ident_d)
    bias_sb = const_pool.tile([P, 3 * CCH], F32, tag="bias")
    nc.sync.dma_start(bias_sb[:], bias_d)
    # A pre-scaled by ASC, zero-padded to 1024 rows on host.
    a_sb = big_pool.tile([P, CG8, 2, C], FP8, tag="a")
    for bb in range(2 * CG8):
        nc.sync.dma_start(a_sb[:, bb // 2, bb % 2, :],
                          a_d[bb * P:(bb + 1) * P, :])
    # tail weights, cast to bf16 on DMA; queued behind the x loads so they
    # land long before the tail needs them
    w3 = big_pool.tile([P, 3, CCH, C], BF16, tag="w3")

    # --- persistent x representations ---
    xbf = big_pool.tile([P, NT, C], BF16, tag="xbf")
    xt = big_pool.tile([P, CG8, 2, N], FP8, tag="xt")
    gt = big_pool.tile([P, CG8, 2, N], FP8, tag="gt")
    nc.vector.memset(xt[:, CG8 - 1, 1, :], 0.0)   # pad features 896..1023
    nc.vector.memset(gt[:, CG8 - 1, 1, :], 0.0)

    # --- phase 0: stream x in, transpose, project G per 1024-token chunk ---
    for ch in range(NCH):
        for q in range(TCHM // P):
            nt = ch * (TCHM // P) + q
            nc.gpsimd.dma_start(xbf[:, nt, :], x_d[nt * P:(nt + 1) * P, :])
            for cc in range(CCH):
                pt = ps_pool.tile([P, P], BF16, tag="ps", name="pt")
                nc.tensor.transpose(pt[:], xbf[:, nt, cc * P:(cc + 1) * P],
                                    ident[:])
                dst = xt[:, cc // 2, cc % 2, nt * P:(nt + 1) * P]
                if cc % 2 == 0:
                    nc.vector.tensor_copy(dst, pt[:])
                else:
                    nc.scalar.copy(dst, pt[:])
        for dd in range(CCH):
            gp = ps_pool.tile([P, 2, TCH], F32, tag="ps", name="gp")
            for g in range(CG8):
                for h2 in range(2):
                    nc.tensor.matmul(
                        gp[:, h2, :], a_sb[:, g, :, dd * P:(dd + 1) * P],
                        xt[:, g, :,
                           ch * TCHM + h2 * TCH:ch * TCHM + (h2 + 1) * TCH],
                        start=(g == 0), stop=(g == CG8 - 1),
                        perf_mode=DR, skip_group_check=True,
                    )
            dst = gt[:, dd // 2, dd % 2, ch * TCHM:(ch + 1) * TCHM]
            if dd % 2 == 0:
                nc.vector.tensor_copy(dst, gp[:])
            else:
                nc.scalar.copy(dst, gp[:])
    for i in range(3):
        wk = ("wvp", "wf1", "wf2")[i]
        nc.gpsimd.dma_start(w3[:, i, :, :],
                            aps[wk].rearrange("(cc p) e -> p cc e", p=P))

    # --- main loop: S -> exp -> weighted colsum, per pair of row tiles ---
    NPAIR = NT // 2
    tT = acc_pool.tile([P, NT], F32, tag="acc")  # t * N accumulator
    pend = []  # deferred colsum emission: (pair, e2, rf2)

    def emit_colsum(pair, e2, rf2):
        for j in range(NT):
            nc.tensor.matmul(
                tT[:, j:j + 1], e2[:, :, j * P:(j + 1) * P], rf2[:],
                start=(pair == 0 and j == 0),
                stop=(pair == NPAIR - 1 and j == NT - 1),
                perf_mode=DR, skip_group_check=True,
            )

    for pair in range(NPAIR):
        e2 = e_pool.tile([P, 2, N], FP8, tag="e2")
        rf2 = small_pool.tile([P, 2, 1], FP8, tag="rf2")
        for h in range(2):
            nt = 2 * pair + h
            zp = small_pool.tile([P, NMC // 2], F32, tag="zp")
            for mj2 in range(NMC // 2):
                sps = ps_pool.tile([P, 2, MCH], F32, tag="ps", name="sps")
                for g in range(CG8):
                    for k2 in range(2):
                        mj = 2 * mj2 + k2
                        nc.tensor.matmul(
                            sps[:, k2, :], gt[:, g, :, nt * P:(nt + 1) * P],
                            xt[:, g, :, mj * MCH:(mj + 1) * MCH],
                            start=(g == 0), stop=(g == CG8 - 1),
                            perf_mode=DR, skip_group_check=True,
                        )
                nc.scalar.activation(
                    e2[:, h, mj2 * 2 * MCH:(mj2 + 1) * 2 * MCH], sps[:],
                    mybir.ActivationFunctionType.Exp,
                    scale=1.0 / ASC,
                    accum_out=zp[:, mj2:mj2 + 1],
                )
            # PE keeps streaming the next tile's S while this runs on DVE
            if h == 0 and pend:
                emit_colsum(*pend.pop(0))
            z = small_pool.tile([P, 1], F32, tag="z")
            nc.vector.reduce_sum(z[:], zp[:], axis=mybir.AxisListType.X)
            zn = small_pool.tile([P, 1], F32, tag="zn")
            nc.vector.tensor_scalar_mul(zn[:], z[:], 1.0 / N)
            # r = N/Z ~ O(1); fp8 quantization of r is ~6% per row but the
            # colsum averages 4096 independent rows -> ~0.1% on t
            with nc.allow_low_precision("r stat-averages over 4096 rows"):
                nc.vector.reciprocal(rf2[:, h, :], zn[:])
        pend.append((pair, e2, rf2))
    while pend:
        emit_colsum(*pend.pop(0))

    # --- tail ---
    # TO[:, j, 0] = (t*N)[j-chunk] / N^2 = t/N ; TO[:, j, 1] = 1/N
    TO = tail_pool.tile([P, NT, 2], BF16, tag="to")
    nc.vector.memset(TO[:, :, 1], 1.0 / N)
    nc.scalar.mul(TO[:, :, 0], tT[:], 1.0 / (N * N))

    # Y2[0, c] = u = (t/N) @ x ; Y2[1, c] = xbar (mean over tokens)
    Y2 = ps_pool.tile([2, C], F32, tag="ps", name="y2")
    for j in range(NT):
        for (o, w) in ((0, 512), (512, 384)):
            nc.tensor.matmul(
                Y2[:, o:o + w], TO[:, j, :], xbf[:, j, o:o + w],
                start=(j == 0), stop=(j == NT - 1),
                skip_group_check=True,
            )
    y2S = tail_pool.tile([2, C], F32, tag="y2s")
    nc.scalar.copy(y2S[:], Y2[:])
    identF2 = tail_pool.tile([2, 2], F32, tag="idf2")
    nc.vector.tensor_copy(identF2[:], ident[0:2, 0:2])
    uxS = tail_pool.tile([P, CCH, 2], F32, tag="ux")
    for cc in range(CCH):
        ptx = ps_pool.tile([P, 2], F32, tag="ps", name="ptx")
        nc.tensor.transpose(ptx[:], y2S[:, cc * P:(cc + 1) * P], identF2[:])
        nc.vector.tensor_copy(uxS[:, cc, :], ptx[:])

    def matvec(wi, vec_cols, out_psum):
        for ee in range(CCH):
            for cc in range(CCH):
                nc.tensor.matmul(
                    out_psum[:, ee:ee + 1],
                    w3[:, wi, cc, ee * P:(ee + 1) * P],
                    vec_cols(cc),
                    start=(cc == 0), stop=(cc == CCH - 1),
                )

    uxB = tail_pool.tile([P, CCH, 1], BF16, tag="uxb")
    nc.vector.tensor_copy(uxB[:], uxS[:, :, 0:1])
    P2 = ps_pool.tile([P, CCH], F32, tag="ps", name="p2")
    matvec(0, lambda cc: uxB[:, cc, :], P2)
    pooledS = tail_pool.tile([P, CCH], F32, tag="pooled")
    nc.vector.tensor_add(pooledS[:], P2[:], uxS[:, :, 1])
    nc.vector.tensor_add(pooledS[:], pooledS[:], bias_sb[:, 0:CCH])
    pooledB = tail_pool.tile([P, CCH], BF16, tag="pooledb")
    nc.vector.tensor_copy(pooledB[:], pooledS[:])

    H2 = ps_pool.tile([P, CCH], F32, tag="ps", name="h2")
    matvec(1, lambda cc: pooledB[:, cc:cc + 1], H2)
    hS = tail_pool.tile([P, CCH], F32, tag="h")
    nc.vector.tensor_add(hS[:], H2[:], bias_sb[:, CCH:2 * CCH])
    nc.vector.tensor_scalar_max(hS[:], hS[:], 0.0)
    hB = tail_pool.tile([P, CCH], BF16, tag="hb")
    nc.vector.tensor_copy(hB[:], hS[:])

    O2 = ps_pool.tile([P, CCH], F32, tag="ps", name="o2")
    matvec(2, lambda cc: hB[:, cc:cc + 1], O2)
    outS = tail_pool.tile([P, CCH], F32, tag="out")
    nc.vector.tensor_add(outS[:], O2[:], bias_sb[:, 2 * CCH:3 * CCH])
    nc.sync.dma_start(out_d, outS[:])


_NC_CACHE = {}


def build_nc(reps=1):
    key = ("nc", reps)
    if key in _NC_CACHE:
        return _NC_CACHE[key]
    nc = bacc.Bacc(
        "TRN2", target_bir_lowering=False, debug=False,
        enable_asserts=False, num_devices=NCORES,
    )
    aps = {
        "xc": nc.dram_tensor("xc", [N, C], F32, kind="ExternalInput").ap(),
        "abf": nc.dram_tensor("abf", [2 * CG8 * P, C], FP8,
                              kind="ExternalInput").ap(),
        "wvp": nc.dram_tensor("wvp", [C, C], F32, kind="ExternalInput").ap(),
        "wf1": nc.dram_tensor("wf1", [C, C], F32, kind="ExternalInput").ap(),
        "wf2": nc.dram_tensor("wf2", [C, C], F32, kind="ExternalInput").ap(),
        "biasR": nc.dram_tensor("biasR", [P, 3 * CCH], F32,
                                kind="ExternalInput").ap(),
        "ident": nc.dram_tensor("ident", [P, P], BF16,
                                kind="ExternalInput").ap(),
        "outT": nc.dram_tensor("outT", [P, CCH], F32,
                               kind="ExternalOutput").ap(),
    }
    with tile.TileContext(nc) as tc:
        for _ in range(reps):
            with ExitStack() as ctx:
                _build_body(ctx, tc, aps)
    nc.compile()
    _NC_CACHE[key] = nc
    return nc


def prep_in_maps(x_, Wq, Wk, Wv, Wp, bp, Wf1, bf1, Wf2, bf2):
    f32 = np.float32
    x_ = np.ascontiguousarray(np.asarray(x_, dtype=f32))
    A = (np.asarray(Wq, f32) @ np.asarray(Wk, f32).T) * np.float32(C ** -0.5)
    a_pad = np.zeros((2 * CG8 * P, C), f32)
    a_pad[:C] = A * np.float32(ASC)
    abf = np.ascontiguousarray(a_pad.astype(_F8))
    wvp = np.ascontiguousarray(np.asarray(Wv, f32) @ np.asarray(Wp, f32))
    wf1 = np.ascontiguousarray(np.asarray(Wf1, f32))
    wf2 = np.ascontiguousarray(np.asarray(Wf2, f32))
    biasR = np.concatenate(
        [np.asarray(b, f32).reshape(CCH, P).T for b in (bp, bf1, bf2)], axis=1
    )
    biasR = np.ascontiguousarray(biasR)
    ident = np.eye(P, dtype=_BF)
    shared = {
        "abf": abf, "wvp": wvp, "wf1": wf1, "wf2": wf2,
        "biasR": biasR, "ident": ident,
    }
    return [dict(shared, xc=np.ascontiguousarray(x_[b])) for b in range(B)]


def assemble_output(results):
    out = np.empty((B, C), dtype=np.float32)
    for b in range(B):
        out[b] = np.asarray(results[b]["outT"], np.float32).T.reshape(C)
    return out


def kernel(**inputs) -> np.ndarray:
    nc = build_nc()
    in_maps = prep_in_maps(**inputs)
    res = run_bass_kernel_spmd(nc, in_maps, list(range(NCORES)))
    return assemble_output(res.results)


if __name__ == "__main__":
    import reference as R
    inp = {k: np.asarray(v) for k, v in R.setup_inputs().items()}
    out = kernel(**inp)
    print(out.shape, out.dtype)
